# revision 12
# baseline (speedup 1.0000x reference)
"""Trainium2 Bass kernel: single-head causal self-attention.

Problem: x[B=8, S=2048, D=1024], Wq/Wk/Wv[D, H=64], bq/bk/bv[H].
    q = x@Wq+bq; k = x@Wk+bk; v = x@Wv+bv
    out = softmax(causal(q k^T) / sqrt(H)) @ v

Sharding: batch -> 8 NeuronCores (data parallel, no collectives).

Per-core strategy (v3 — PE-roofline oriented):
  - x shard is host-transposed to bf16 [D, S] chunks, loaded via the
    sync + gpsimd DMA queues (scalar queue is reserved for exp). The
    first q-chunk is split into pieces so the first projection matmul
    gates on ~256 KiB; later chunks are whole-chunk transfers.
  - all matmuls bf16: Wq|Wk packed [D,128] stationary; Wv carries an
    appended zero column whose bias is 1.0, so P @ [V|1] yields the
    softmax denominator as an extra output column
  - scores are built transposed: S^T [128 k, 512 q] = K_i Q^T in PSUM,
    exp (scale=1/8, bf16 out) over two PSUM banks at a time gives P^T
    tiles in SBUF = the moving operand of O^T = (V|1)^T P^T
  - causal structure exploited at 128-block granularity: diagonal-block
    score and PV matmuls skip the fully-masked q-range; only the
    128x128 triangle block needs a multiplicative 0/1 mask (on GpSimd)
  - software pipelining: PE order is proj_J -> out_{J-1} -> att_J so
    the out-phase vector work overlaps projection matmuls
  - O^T [65, 512] chunks are PE-transposed back (bf16), normalized by
    the reciprocal of the ones-column, and DMA'd out per q-chunk
"""

import sys

sys.path.insert(0, "/opt/trn_rl_repo")

import numpy as np

B, S, D, H = 8, 2048, 1024, 64
N_CORES = 8
SQ = 512            # q chunk (PSUM bank / fp32 moving max)
NQ = S // SQ        # 4
ND = D // 128       # 8 contraction chunks for projections
H1 = H + 1          # V plus ones column
TRIM = True         # skip fully-masked q-ranges in diagonal blocks

_CACHE = {}


def _build_nc():
    import concourse.tile as tile
    from concourse import bacc, mybir

    f32 = mybir.dt.float32
    bf16 = mybir.dt.bfloat16
    AF = mybir.ActivationFunctionType

    nc = bacc.Bacc(None, target_bir_lowering=False)
    xTp = nc.dram_tensor("xTp", [NQ, 128, ND * SQ], bf16, kind="ExternalInput")
    wqk = nc.dram_tensor("wqk", [128, ND * 2 * H], bf16, kind="ExternalInput")
    wv1 = nc.dram_tensor("wv1", [128, ND * H1], bf16, kind="ExternalInput")
    bqk = nc.dram_tensor("bqk", [2 * H, 1], f32, kind="ExternalInput")
    bv1 = nc.dram_tensor("bv1", [H1, 1], f32, kind="ExternalInput")
    # maskb[kk, v] = (v - 384 >= kk); triangle = maskb[:, 384:512],
    # span mask for block r = maskb[:, 384-128r : 512]
    maskb = nc.dram_tensor("maskb", [128, SQ], bf16, kind="ExternalInput")
    identb = nc.dram_tensor("identb", [128, 128], bf16, kind="ExternalInput")
    out = nc.dram_tensor("out", [S, H], f32, kind="ExternalOutput")

    with tile.TileContext(nc) as tc:
        from contextlib import ExitStack

        with ExitStack() as ctx:
            const = ctx.enter_context(tc.tile_pool(name="const", bufs=1))
            sb = ctx.enter_context(tc.tile_pool(name="sb", bufs=1))
            pt_pool = ctx.enter_context(tc.tile_pool(name="pt", bufs=4))
            o_pool = ctx.enter_context(tc.tile_pool(name="o", bufs=2))
            ps = ctx.enter_context(tc.tile_pool(name="ps", bufs=1, space="PSUM"))

            wqk_sb = const.tile([128, ND * 2 * H], bf16)
            wv1_sb = const.tile([128, ND * H1], bf16)
            bqk_sb = const.tile([128, 1], f32)
            bv1_sb = const.tile([H1, 1], f32)
            maskb_sb = const.tile([128, SQ], bf16)
            identb_sb = const.tile([128, 128], bf16)

            xt = {}          # J -> [128, ND*SQ] bf16 (c-chunk at cols c*SQ)
            QKT = {}         # J -> [128, SQ] bf16 (Q^T rows 0:64, K^T 64:128)
            KT0 = {}         # J -> [64, SQ] bf16 at base partition 0
            Vones = {}       # t -> [128, 4*66] bf16 ((V|1) rows, stride 66)
            VT1 = {}         # J -> [H1, SQ] bf16 (V^T plus ones row)

            # ---- DMA issue order defines per-queue FIFO order ----
            # sync:   wqk, J0 pieces c0-c5, J1..J3 whole chunks,
            #         (kt0 copies + out stores later)
            # scalar: NOTHING (exp only — dispatch backlog delays exp)
            # gpsimd: small consts, wv1, J0 pieces c6-c7, maskb,
            #         (triangle masks later)
            with nc.named_scope("load"):
                for J in range(NQ):
                    t_x = sb.tile([128, ND * SQ], bf16, tag=f"x{J}")
                    xt[J] = t_x
                nc.sync.dma_start(wqk_sb[:], wqk[:, :])
                nc.gpsimd.dma_start(bqk_sb[:], bqk[:, :])
                nc.gpsimd.dma_start(bv1_sb[:], bv1[:, :])
                nc.gpsimd.dma_start(identb_sb[:], identb[:, :])
                nc.gpsimd.dma_start(wv1_sb[:], wv1[:, :])
                for c0 in range(0, 6, 2):
                    nc.sync.dma_start(
                        xt[0][:, c0 * SQ : (c0 + 2) * SQ],
                        xTp[0, :, c0 * SQ : (c0 + 2) * SQ],
                    )
                nc.gpsimd.dma_start(
                    xt[0][:, 6 * SQ :], xTp[0, :, 6 * SQ :]
                )
                nc.gpsimd.dma_start(maskb_sb[:], maskb[:, :])
                for J in range(1, NQ):
                    nc.sync.dma_start(xt[J][:], xTp[J, :, :])

            def proj(J):
                with nc.named_scope(f"proj{J}"):
                    qk = ps.tile([128, SQ], f32, tag="proj", bufs=2)
                    for c in range(ND):
                        nc.tensor.matmul(
                            qk[:],
                            wqk_sb[:, c * 2 * H : (c + 1) * 2 * H],
                            xt[J][:, c * SQ : (c + 1) * SQ],
                            start=(c == 0),
                            stop=(c == ND - 1),
                        )
                    qkt = sb.tile([128, SQ], bf16, tag=f"qkt{J}")
                    nc.vector.tensor_scalar_add(qkt[:], qk[:], bqk_sb[:])
                    QKT[J] = qkt
                    kt0 = sb.tile([H, SQ], bf16, tag=f"kt0{J}")
                    nc.sync.dma_start(kt0[:], qkt[H : 2 * H, :])
                    KT0[J] = kt0

                    vv = ps.tile([H1, SQ], f32, tag="proj", bufs=2)
                    for c in range(ND):
                        nc.tensor.matmul(
                            vv[:],
                            wv1_sb[:, c * H1 : (c + 1) * H1],
                            xt[J][:, c * SQ : (c + 1) * SQ],
                            start=(c == 0),
                            stop=(c == ND - 1),
                        )
                    vt1 = sb.tile([H1, SQ], bf16, tag=f"vt1{J}")
                    nc.vector.tensor_scalar_add(vt1[:], vv[:], bv1_sb[:])
                    VT1[J] = vt1

            def vtrans(J):
                # V~ = (V|1) in [s, h'] rows via PE transposes, all four
                # into one PSUM tile -> one SBUF copy per q-chunk
                # stride 66 keeps each bf16 PSUM output 4-byte aligned
                with nc.named_scope(f"vtr{J}"):
                    vt1 = VT1[J]
                    pst = ps.tile([128, 4 * 66], bf16, tag="vtr", bufs=1)
                    for tt in range(4):
                        nc.tensor.transpose(
                            pst[:, tt * 66 : tt * 66 + H1],
                            vt1[:, tt * 128 : (tt + 1) * 128],
                            identb_sb[:H1, :H1],
                        )
                    vo = sb.tile([128, 4 * 66], bf16, tag=f"vo{J}")
                    nc.vector.tensor_copy(
                        vo[:].rearrange("p (t u) -> p t u", t=4)[:, :, 0:H1],
                        pst[:].rearrange("p (t u) -> p t u", t=4)[:, :, 0:H1],
                    )
                    Vones[J] = vo

            OT = {}

            def att(J):
                with nc.named_scope(f"att{J}"):
                    ot = ps.tile([H1, SQ], f32, tag="ot", bufs=1)
                    OT[J] = ot
                    nhalf = 2 * (J + 1)   # pairs of k-chunks
                    for ii in range(nhalf):
                        diag = ii >= 2 * J
                        st = ps.tile([128, 2 * SQ], f32, tag="st", bufs=2)
                        for h2 in range(2):
                            i = 2 * ii + h2
                            r = i - 4 * J
                            q0 = 128 * r if (TRIM and diag and r > 0) else 0
                            nc.tensor.matmul(
                                st[:, h2 * SQ + q0 : (h2 + 1) * SQ],
                                KT0[i // 4][:, (i % 4) * 128 : (i % 4 + 1) * 128],
                                QKT[J][:H, q0:],
                                start=True,
                                stop=True,
                            )
                        pt = pt_pool.tile([128, 2 * SQ], bf16, tag="pt")
                        nc.scalar.activation(pt[:], st[:], AF.Exp, scale=0.125)
                        if diag:
                            for h2 in range(2):
                                r = 2 * ii + h2 - 4 * J
                                if TRIM:
                                    # only the 128x128 triangle block needs
                                    # masking; fully-masked cols are skipped
                                    # by the trimmed PV matmuls below
                                    nc.gpsimd.tensor_mul(
                                        pt[:, h2 * SQ + 128 * r :
                                           h2 * SQ + 128 * (r + 1)],
                                        pt[:, h2 * SQ + 128 * r :
                                           h2 * SQ + 128 * (r + 1)],
                                        maskb_sb[:, 384:],
                                    )
                                else:
                                    span = 128 * (r + 1)
                                    nc.gpsimd.tensor_mul(
                                        pt[:, h2 * SQ : h2 * SQ + span],
                                        pt[:, h2 * SQ : h2 * SQ + span],
                                        maskb_sb[:, 384 - 128 * r :],
                                    )
                        for h2 in range(2):
                            i = 2 * ii + h2
                            r = i - 4 * J
                            q0 = 128 * r if (TRIM and diag and r > 0) else 0
                            nc.tensor.matmul(
                                ot[:, q0:],
                                Vones[i // 4][:, (i % 4) * 66 : (i % 4) * 66 + H1],
                                pt[:, h2 * SQ + q0 : (h2 + 1) * SQ],
                                start=(i == 0),
                                stop=(i == 4 * (J + 1) - 1),
                            )

            def outp(J):
                # normalize + store rows 512J..512J+511
                with nc.named_scope(f"out{J}"):
                    ot = OT[J]
                    ots = sb.tile([H1, SQ], bf16, tag=f"ots{J}")
                    nc.vector.tensor_copy(ots[:], ot[:])
                    ob = o_pool.tile([128, 4 * H], f32, tag="ob")
                    po = ps.tile([128, 4 * 66], bf16, tag="vtr", bufs=1)
                    for tt in range(4):
                        nc.tensor.transpose(
                            po[:, tt * 66 : tt * 66 + H1],
                            ots[:, tt * 128 : (tt + 1) * 128],
                            identb_sb[:H1, :H1],
                        )
                    rc = o_pool.tile([128, 4], f32, tag="rc")
                    nc.vector.reciprocal(rc[:], po[:, H :: 66])
                    for tt in range(4):
                        nc.vector.tensor_scalar_mul(
                            ob[:, tt * H : (tt + 1) * H],
                            po[:, tt * 66 : tt * 66 + H],
                            rc[:, tt : tt + 1],
                        )
                    nc.sync.dma_start(
                        out[J * SQ : (J + 1) * SQ, :].rearrange(
                            "(t p) h -> p t h", p=128
                        ),
                        ob[:].rearrange("p (t h) -> p t h", t=4),
                    )

            # software-pipelined schedule: out_{J-1} PE-transposes overlap
            # proj_J matmuls' vector work; att_J starts with kt0_J ready
            proj(0)
            vtrans(0)
            att(0)
            for J in range(1, NQ):
                proj(J)
                outp(J - 1)
                vtrans(J)
                att(J)
            outp(NQ - 1)

    nc.finalize()
    return nc


def _host_prep(x, Wq, bq, Wk, bk, Wv, bv):
    """Layout-only host prep: shard x by batch + pack weight operands."""
    import ml_dtypes

    f32 = np.float32
    bf16 = ml_dtypes.bfloat16
    wqk = np.concatenate([Wq, Wk], axis=1)          # [D, 128]
    # pack [D, M] -> [128, ND*M]: chunk c of 128 D-rows at cols c*M..
    wqk = np.ascontiguousarray(
        wqk.reshape(ND, 128, 2 * H).transpose(1, 0, 2).reshape(128, ND * 2 * H),
        dtype=bf16,
    )
    wv1 = np.concatenate([Wv, np.zeros((D, 1), f32)], axis=1)  # [D, 65]
    wv1 = np.ascontiguousarray(
        wv1.reshape(ND, 128, H1).transpose(1, 0, 2).reshape(128, ND * H1),
        dtype=bf16,
    )
    bqk = np.ascontiguousarray(np.concatenate([bq, bk])[:, None], dtype=f32)
    bv1 = np.ascontiguousarray(
        np.concatenate([bv, np.ones((1,), f32)])[:, None], dtype=f32
    )
    # maskb[kk, v] = (v - 384 >= kk): block r's span mask (keep
    # qq >= 128r + kk over qq in [0, 128(r+1))) is maskb[:, 384-128r:512];
    # the shared 128x128 triangle is maskb[:, 384:512]
    kk = np.arange(128)[:, None]
    vv_ = np.arange(SQ)[None, :]
    maskb = (vv_ - 384 >= kk).astype(bf16)
    identb = np.eye(128, dtype=bf16)
    common = {
        "wqk": wqk, "wv1": wv1, "bqk": bqk, "bv1": bv1,
        "maskb": maskb, "identb": identb,
    }
    in_maps = []
    for b in range(B):
        m = dict(common)
        # xTp[J, p, c*SQ+s] = x[b][SQ*J+s, 128*c+p]
        m["xTp"] = np.ascontiguousarray(
            x[b].reshape(NQ, SQ, ND, 128).transpose(0, 3, 2, 1), dtype=bf16
        ).reshape(NQ, 128, ND * SQ)
        in_maps.append(m)
    return in_maps


def run(x, Wq, bq, Wk, bk, Wv, bv, trace=False):
    from concourse.bass_utils import run_bass_kernel_spmd

    if "nc" not in _CACHE:
        _CACHE["nc"] = _build_nc()
    nc = _CACHE["nc"]
    in_maps = _host_prep(
        np.asarray(x), np.asarray(Wq), np.asarray(bq), np.asarray(Wk),
        np.asarray(bk), np.asarray(Wv), np.asarray(bv),
    )
    res = run_bass_kernel_spmd(
        nc, in_maps, core_ids=list(range(N_CORES)), trace=trace
    )
    outs = np.stack([res.results[c]["out"] for c in range(N_CORES)], axis=0)
    return outs.astype(np.float32), res


def kernel(x, Wq, bq, Wk, bk, Wv, bv):
    outs, _ = run(x, Wq, bq, Wk, bk, Wv, bv, trace=False)
    return outs
